# revision 36
# baseline (speedup 1.0000x reference)
"""Trainium2 Bass kernel for nn_ExactScalarArray.

Math: the reference computes, per (b, l):  prod_k reduce(c1*c2, p1+p2)
in an exact ring representation Z[w], w = e^{i pi/4}, then converts to
complex and sums over l with power-of-two alignment.  The ring embed
into C is a homomorphism and the reduce step is value-preserving, so
the whole thing equals

    out[b] = sum_l ( prod_k v1(b,l,k) * v2(b,l,k) ) * 2^{S(b,l)}
    v(c)   = (c0 + (c1+c3)/sqrt2) + i (c2 + (c1-c3)/sqrt2)
    S      = sum_k (p1+p2)

evaluated here in f32 complex arithmetic (max rel err vs the reference
~9e-6, measured).  Sharding: batch dim B=256 split across 8 cores; all
reduction axes (K, L) are core-local, so no collectives.

Host-side, the inputs (exact {0,1} values) are packed into dense bf16
component blocks inside one tensor per core: lossless, halves the HBM
traffic, and each chunk is a single DMA (one wait on the first
consumer; each ISA instruction has one sync-wait slot).
"""

import numpy as np

import concourse.bass as bass
import concourse.mybir as mybir
import concourse.tile as tile
from concourse.bass_utils import run_bass_kernel_spmd

# Problem shape (hardcoded per contract)
B, L, K = 256, 2048, 8
NCORES = 8
BC = B // NCORES            # 32 batch rows per core
NR = BC * L                 # 65536 (b,l) rows per core
P = 128                     # SBUF partitions
RPP = NR // P               # 512 rows per partition
TC = 256                    # rows-per-partition per chunk
NCHUNK = RPP // TC          # 2 chunks
NBLK = 12                   # bf16 blocks per chunk (each TC*K):
                            #   c1_1, c1_3, -c1_3, c1_0, c1_2,
                            #   c2_1, c2_3, -c2_3, c2_0, c2_2, p1, p2
                            # negated blocks exist because the DMA CCE
                            # accumulator only has ADD
INV_SQRT2 = 0.7071067811865476

FP = mybir.dt.float32
BF = mybir.dt.bfloat16
I32 = mybir.dt.int32
ALU = mybir.AluOpType
AX = mybir.AxisListType

FK = TC * K               # one block, bf16 elements
FXF = TC * K * NBLK // 2  # packed chunk size in f32 units


def build_program(split_waits=True):
    nc = bass.Bass("TRN2", target_bir_lowering=False, debug=False,
                   num_devices=NCORES)
    xind = nc.dram_tensor("xin", [P, RPP * K * NBLK // 2], FP,
                          kind="ExternalInput").ap()
    outd = nc.dram_tensor("out", [P, 2], FP, kind="ExternalOutput").ap()
    with tile.TileContext(nc) as tc:
        build_kernel(nc, tc, xind, outd)
    if split_waits:
        _split_multiwait(nc)
    return nc


def _split_multiwait(nc):
    """Walrus allows one sync-wait per ISA instruction; hoist extras onto
    NOPs inserted just before the offender on the same engine."""
    k = 0
    for f in nc.m.functions:
        for bb in f.blocks:
            il = bb.instructions
            i = 0
            while i < len(il):
                inst = il[i]
                si = inst.sync_info
                if si is not None and si.on_wait and len(si.on_wait) > 1:
                    waits = list(si.on_wait)
                    for w in waits[:-1]:
                        nop = mybir.InstNoOp(name=f"WSPLIT-{k}", ins=[], outs=[])
                        k += 1
                        nop.engine = inst.engine
                        nop.sync_info = mybir.SyncInfo(on_wait=[w], on_update=[])
                        il.insert(i, nop)
                        i += 1
                    si.on_wait = waits[-1:]
                    inst.sync_info = si
                i += 1


def build_kernel(nc, tc, xind, outd):
    # With NCHUNK == 2 and double-buffered io/head pools there is no SBUF
    # slot reuse across chunks for the DMA'd tile or its first consumer, so
    # no instruction needs two sync waits (one ISA wait slot each).  Work
    # tiles are tag-aliased to fit SBUF; every alias chain is read/written
    # strictly sequentially on the Vector engine.
    with (
        tc.tile_pool(name="io", bufs=2) as io_pool,
        tc.tile_pool(name="head", bufs=2) as head_pool,
        tc.tile_pool(name="work", bufs=1) as work_pool,
        tc.tile_pool(name="acc", bufs=1) as acc_pool,
    ):
        acc = acc_pool.tile([P, 2 * NCHUNK], FP)

        for ch in range(NCHUNK):
            # The DMA engines' inline CCE adder computes t1 = c1+c3,
            # t2 = c1+(-c3) and ps = p1+p2 during the loads: base copy, then
            # a second DMA with accum_op=add onto the same half.  The
            # sync-wait split pass legalizes readers that see two writer
            # semaphores.
            base = ch * FXF

            def bsrc(i, n=1):
                half = FK // 2  # one block in f32 units
                return xind[:, base + i * half:base + (i + n) * half].bitcast(BF)

            tj1 = head_pool.tile([P, 2 * FK], BF, tag="tj1")
            tj2 = head_pool.tile([P, 2 * FK], BF, tag="tj2")
            ps = work_pool.tile([P, FK], BF, tag="ps")
            xb = io_pool.tile([P, FK], FP, tag="xb")
            xd = io_pool.tile([P, FK], FP, tag="xd")

            # Issue order keeps the Pool FIFO from stalling: each CCE
            # accumulate waits on its base copy's completion, so put
            # independent transfers between a base and its accumulate.
            nc.gpsimd.dma_start(tj1[:, 0:FK], bsrc(0))
            nc.gpsimd.dma_start(tj1[:, FK:2 * FK], bsrc(0))
            nc.gpsimd.dma_start(xb[:, :], xind[:, base + 3 * FK // 2:
                                               base + 5 * FK // 2])
            nc.gpsimd.dma_start(tj1[:, 0:FK], bsrc(1), accum_op=ALU.add)
            nc.gpsimd.dma_start(tj1[:, FK:2 * FK], bsrc(2), accum_op=ALU.add)
            nc.gpsimd.dma_start(tj2[:, 0:FK], bsrc(5))
            nc.gpsimd.dma_start(tj2[:, FK:2 * FK], bsrc(5))
            nc.gpsimd.dma_start(xd[:, :], xind[:, base + 8 * FK // 2:
                                               base + 10 * FK // 2])
            nc.gpsimd.dma_start(tj2[:, 0:FK], bsrc(6), accum_op=ALU.add)
            nc.gpsimd.dma_start(tj2[:, FK:2 * FK], bsrc(7), accum_op=ALU.add)
            nc.gpsimd.dma_start(ps[:, :], bsrc(10))
            nc.gpsimd.dma_start(ps[:, :], bsrc(11), accum_op=ALU.add)

            touches = {}
            for nm, t in (("tj1", tj1), ("tj2", tj2), ("xb", xb),
                          ("xd", xd), ("ps", ps)):
                scr = head_pool.tile([P, 1], t.dtype, tag=f"scr_{nm}")
                touches[nm] = nc.vector.tensor_copy(scr[:, :], t[:, 0:1])

            def after_touch(inst, nm):
                tile.add_dep_helper(inst.ins, touches[nm].ins, False,
                                    "toucher carries the DMA wait")
                return inst

            # complexify: v = [re|im], re = c0 + s*t1, im = c2 + s*t2
            v1 = work_pool.tile([P, 2 * FK], FP, tag="v1")
            v2 = work_pool.tile([P, 2 * FK], FP, tag="v2")
            after_touch(nc.vector.scalar_tensor_tensor(
                v1[:, :], tj1[:, :], INV_SQRT2, xb[:, :].bitcast(BF),
                ALU.mult, ALU.add), "tj1")
            after_touch(nc.vector.scalar_tensor_tensor(
                v2[:, :], tj2[:, :], INV_SQRT2, xd[:, :].bitcast(BF),
                ALU.mult, ALU.add), "tj2")

            # powers: S = sum_k (p1+p2) via strided add tree (bf16-exact)
            pk = ps
            kwidth = FK
            first_pk = True
            while kwidth > TC:
                kwidth //= 2
                nk = work_pool.tile([P, kwidth], BF, tag=f"pk{kwidth}")
                inst = nc.vector.tensor_tensor(
                    nk[:, :], pk[:, 0:2 * kwidth:2], pk[:, 1:2 * kwidth:2],
                    ALU.add)
                if first_pk:
                    after_touch(inst, "ps")
                    first_pk = False
                pk = nk
            S_t = pk

            # pairwise product w = v1*v2, as [re|im] halves
            m12 = work_pool.tile([P, 2 * FK], FP, tag="m12")
            m34 = work_pool.tile([P, 2 * FK], FP, tag="m34")
            # v2 swapped halves: [im2|re2]
            v2sw = v2[:, :].rearrange("p (two n) -> p two n", two=2)[:, ::-1, :]
            nc.vector.tensor_tensor(m12[:, :], v1[:, :], v2[:, :], ALU.mult)
            nc.vector.tensor_tensor(m34[:, :], v1[:, :], v2sw, ALU.mult)
            w = work_pool.tile([P, 2 * FK], FP, tag="v1")  # v1 dead now
            nc.vector.tensor_tensor(
                w[:, 0:FK], m12[:, 0:FK], m12[:, FK:2 * FK], ALU.subtract)
            nc.vector.tensor_tensor(
                w[:, FK:2 * FK], m34[:, 0:FK], m34[:, FK:2 * FK], ALU.add)

            # product tree over K: 8 -> 4 -> 2 -> 1.  w layout [P, 2, width]
            width = FK
            while width > TC:
                width //= 2
                wv = w[:, :].rearrange("p (two n) -> p two n", two=2)
                ev = wv[:, :, 0::2]
                ov = wv[:, :, 1::2]
                ovsw = ov[:, ::-1, :]
                q12 = work_pool.tile([P, 2 * width], FP, tag="m12")
                q34 = work_pool.tile([P, 2 * width], FP, tag="m34")
                nc.vector.tensor_tensor(
                    q12[:, :].rearrange("p (two n) -> p two n", two=2),
                    ev, ov, ALU.mult)
                nc.vector.tensor_tensor(
                    q34[:, :].rearrange("p (two n) -> p two n", two=2),
                    ev, ovsw, ALU.mult)
                nw = work_pool.tile([P, 2 * width], FP, tag="tj2")
                nc.vector.tensor_tensor(
                    nw[:, 0:width], q12[:, 0:width], q12[:, width:2 * width],
                    ALU.subtract)
                nc.vector.tensor_tensor(
                    nw[:, width:2 * width], q34[:, 0:width],
                    q34[:, width:2 * width], ALU.add)
                w = nw

            # pw = 2^S exactly: (S+127)*2^23 is an exact f32 integer; convert
            # to i32 and reinterpret the bits as f32.  Kept on the Vector
            # engine: an ACT-engine detour exposes cross-engine latency at
            # the chunk tail (measured ~5us stall).
            pq = work_pool.tile([P, TC], FP, tag="pq")
            pwi = work_pool.tile([P, TC], I32, tag="pwi")
            nc.vector.tensor_scalar(
                pq[:, :], S_t[:, :], 127.0, float(1 << 23), ALU.add, ALU.mult)
            nc.vector.tensor_copy(pwi[:, :], pq[:, :])
            pw = pwi[:, :].bitcast(FP)

            # sum_l w * 2^S -> this chunk's [re, im] accumulator columns
            # (STT with free-dim accumulate)
            dummy = work_pool.tile([P, 2 * TC], FP, tag="dummy")
            nc.vector.scalar_tensor_tensor(
                dummy[:, 0:TC], w[:, 0:TC], 1.0, pw, ALU.mult, ALU.mult,
                accum_out=acc[:, 2 * ch:2 * ch + 1])
            nc.vector.scalar_tensor_tensor(
                dummy[:, TC:2 * TC], w[:, TC:2 * TC], 1.0, pw, ALU.mult,
                ALU.mult, accum_out=acc[:, 2 * ch + 1:2 * ch + 2])

        outt = acc_pool.tile([P, 2], FP)
        nc.vector.tensor_reduce(
            outt[:, :], acc[:, :].rearrange("p (c two) -> p two c", two=2),
            AX.X, ALU.add)
        # HWDGE for the tiny result store: the SWDGE path costs a ~4us Q7
        # drain on the kernel tail waiting for the HBM write receipt.
        nc.sync.dma_start(outd[:, :], outt[:, :])


_PROGRAM = None


def _get_program():
    global _PROGRAM
    if _PROGRAM is None:
        _PROGRAM = build_program()
    return _PROGRAM


def _to_bf16_bits(a):
    """f32 array of exact small ints -> uint16 bf16 bit patterns."""
    return (np.ascontiguousarray(a, dtype=np.float32).view(np.uint32) >> 16
            ).astype(np.uint16)


def pack_core_input(c1, c2, p1, p2):
    """Pack one core's inputs into [P, RPP*K*NBLK/2] f32 (bf16 bit blocks).

    Rows (b*L+l) map to partition r//RPP, chunk (r%RPP)//TC; within a chunk
    there are NBLK dense bf16 blocks of TC*K values each:
    c1_1, c1_3, c1_0, c1_2, c2_1, c2_3, c2_0, c2_2, p1, p2."""
    u = np.empty((P, NCHUNK, NBLK, TC * K), dtype=np.uint16)

    def comp(c, j):
        return _to_bf16_bits(c[..., j]).reshape(P, NCHUNK, TC * K)

    u[:, :, 0] = comp(c1, 1)
    u[:, :, 1] = comp(c1, 3)
    u[:, :, 2] = _to_bf16_bits(-c1[..., 3]).reshape(P, NCHUNK, TC * K)
    u[:, :, 3] = comp(c1, 0)
    u[:, :, 4] = comp(c1, 2)
    u[:, :, 5] = comp(c2, 1)
    u[:, :, 6] = comp(c2, 3)
    u[:, :, 7] = _to_bf16_bits(-c2[..., 3]).reshape(P, NCHUNK, TC * K)
    u[:, :, 8] = comp(c2, 0)
    u[:, :, 9] = comp(c2, 2)
    u[:, :, 10] = _to_bf16_bits(p1.astype(np.float32)).reshape(P, NCHUNK, TC * K)
    u[:, :, 11] = _to_bf16_bits(p2.astype(np.float32)).reshape(P, NCHUNK, TC * K)
    return u.reshape(P, -1).view(np.float32)


def kernel(coeffs1, coeffs2, power1, power2):
    coeffs1 = np.asarray(coeffs1, dtype=np.float32)
    coeffs2 = np.asarray(coeffs2, dtype=np.float32)
    power1 = np.asarray(power1)
    power2 = np.asarray(power2)
    nc = _get_program()
    in_maps = []
    for ci in range(NCORES):
        sl = slice(ci * BC, (ci + 1) * BC)
        in_maps.append({
            "xin": pack_core_input(coeffs1[sl], coeffs2[sl],
                                   power1[sl], power2[sl]),
        })
    res = run_bass_kernel_spmd(nc, in_maps, core_ids=list(range(NCORES)))
    outs = []
    for ci in range(NCORES):
        o = res.results[ci]["out"]  # [128, 2]
        outs.append(o.reshape(BC, P // BC, 2).sum(axis=1, dtype=np.float32))
    return np.concatenate(outs, axis=0).astype(np.float32)


# revision 37
# speedup vs baseline: 1.1282x; 1.1282x over previous
"""Trainium2 Bass kernel for nn_ExactScalarArray.

Math: the reference computes, per (b, l):  prod_k reduce(c1*c2, p1+p2)
in an exact ring representation Z[w], w = e^{i pi/4}, then converts to
complex and sums over l with power-of-two alignment.  The ring embed
into C is a homomorphism and the reduce step is value-preserving, so
the whole thing equals

    out[b] = sum_l ( prod_k v1(b,l,k) * v2(b,l,k) ) * 2^{S(b,l)}
    v(c)   = (c0 + (c1+c3)/sqrt2) + i (c2 + (c1-c3)/sqrt2)
    S      = sum_k (p1+p2)

evaluated here in f32 complex arithmetic (max rel err vs the reference
~9e-6, measured).  Sharding: batch dim B=256 split across 8 cores; all
reduction axes (K, L) are core-local, so no collectives.

Host-side, the inputs (exact {0,1} values) are packed into dense bf16
component blocks inside one tensor per core: lossless, halves the HBM
traffic, and each chunk is a single DMA (one wait on the first
consumer; each ISA instruction has one sync-wait slot).
"""

import numpy as np

import concourse.bass as bass
import concourse.mybir as mybir
import concourse.tile as tile
from concourse.bass_utils import run_bass_kernel_spmd

# Problem shape (hardcoded per contract)
B, L, K = 256, 2048, 8
NCORES = 8
BC = B // NCORES            # 32 batch rows per core
NR = BC * L                 # 65536 (b,l) rows per core
P = 128                     # SBUF partitions
RPP = NR // P               # 512 rows per partition
TC = 256                    # rows-per-partition per chunk
NCHUNK = RPP // TC          # 2 chunks
NBLK = 10                   # bf16 blocks per chunk: c1_1,c1_3,c1_0,c1_2,
                            #   c2_1,c2_3,c2_0,c2_2, p1, p2  (each TC*K)
INV_SQRT2 = 0.7071067811865476

FP = mybir.dt.float32
BF = mybir.dt.bfloat16
I32 = mybir.dt.int32
ALU = mybir.AluOpType
AX = mybir.AxisListType

FK = TC * K               # one block, bf16 elements
FXF = TC * K * NBLK // 2  # packed chunk size in f32 units


def build_program(split_waits=True):
    nc = bass.Bass("TRN2", target_bir_lowering=False, debug=False,
                   num_devices=NCORES)
    xind = nc.dram_tensor("xin", [P, RPP * K * NBLK // 2], FP,
                          kind="ExternalInput").ap()
    outd = nc.dram_tensor("out", [P, 2], FP, kind="ExternalOutput").ap()
    with tile.TileContext(nc) as tc:
        build_kernel(nc, tc, xind, outd)
    if split_waits:
        _split_multiwait(nc)
    return nc


def _split_multiwait(nc):
    """Walrus allows one sync-wait per ISA instruction; hoist extras onto
    NOPs inserted just before the offender on the same engine."""
    k = 0
    for f in nc.m.functions:
        for bb in f.blocks:
            il = bb.instructions
            i = 0
            while i < len(il):
                inst = il[i]
                si = inst.sync_info
                if si is not None and si.on_wait and len(si.on_wait) > 1:
                    waits = list(si.on_wait)
                    for w in waits[:-1]:
                        nop = mybir.InstNoOp(name=f"WSPLIT-{k}", ins=[], outs=[])
                        k += 1
                        nop.engine = inst.engine
                        nop.sync_info = mybir.SyncInfo(on_wait=[w], on_update=[])
                        il.insert(i, nop)
                        i += 1
                    si.on_wait = waits[-1:]
                    inst.sync_info = si
                i += 1


def build_kernel(nc, tc, xind, outd):
    # With NCHUNK == 2 and double-buffered io/head pools there is no SBUF
    # slot reuse across chunks for the DMA'd tile or its first consumer, so
    # no instruction needs two sync waits (one ISA wait slot each).  Work
    # tiles are tag-aliased to fit SBUF; every alias chain is read/written
    # strictly sequentially on the Vector engine.
    with (
        tc.tile_pool(name="io", bufs=2) as io_pool,
        tc.tile_pool(name="head", bufs=2) as head_pool,
        tc.tile_pool(name="work", bufs=1) as work_pool,
        tc.tile_pool(name="acc", bufs=1) as acc_pool,
    ):
        acc = acc_pool.tile([P, 2 * NCHUNK], FP)

        for ch in range(NCHUNK):
            # five block-pair loads per chunk so compute starts after the
            # first ~1 MiB; each DMA's completion wait is absorbed by a tiny
            # "toucher" copy, so the real consumers only ever carry their
            # single WAR wait (one ISA wait slot per instruction).
            base = ch * FXF
            xtiles = [None] * 5
            touches = [None] * 5
            for i in (0, 2, 1, 3):   # DMA in consumption order (xp via CCE)
                xi = io_pool.tile([P, FK], FP, tag=f"x{i}")
                nc.gpsimd.dma_start(
                    xi[:, :], xind[:, base + i * FK:base + (i + 1) * FK])
                scr = head_pool.tile([P, 1], FP, tag=f"scr{i}")
                touches[i] = nc.vector.tensor_copy(scr[:, :], xi[:, 0:1])
                xtiles[i] = xi[:, :].bitcast(BF)  # [P, 2*FK] bf16
            xa, xbt, xc, xd = xtiles[:4]

            def after_touch(inst, i):
                tile.add_dep_helper(inst.ins, touches[i].ins, False,
                                    "toucher carries the DMA wait")
                return inst

            # complexify both inputs.  t-tiles hold [t1|t2] = [c1+c3|c1-c3]
            # (exact small ints, bf16, 2x DVE mode); v-tiles hold [re|im] f32.
            tj1 = head_pool.tile([P, 2 * FK], BF, tag="tj1")
            tj2 = work_pool.tile([P, 2 * FK], BF, tag="tj2")
            v1 = work_pool.tile([P, 2 * FK], FP, tag="v1")
            v2 = work_pool.tile([P, 2 * FK], FP, tag="v2")
            after_touch(nc.vector.tensor_tensor(
                tj1[:, 0:FK], xa[:, 0:FK], xa[:, FK:2 * FK], ALU.add), 0)
            after_touch(nc.vector.tensor_tensor(
                tj1[:, FK:2 * FK], xa[:, 0:FK], xa[:, FK:2 * FK],
                ALU.subtract), 0)
            after_touch(nc.vector.tensor_tensor(
                tj2[:, 0:FK], xc[:, 0:FK], xc[:, FK:2 * FK], ALU.add), 2)
            after_touch(nc.vector.tensor_tensor(
                tj2[:, FK:2 * FK], xc[:, 0:FK], xc[:, FK:2 * FK],
                ALU.subtract), 2)
            after_touch(nc.vector.scalar_tensor_tensor(
                v1[:, :], tj1[:, :], INV_SQRT2, xbt[:, :], ALU.mult,
                ALU.add), 1)
            after_touch(nc.vector.scalar_tensor_tensor(
                v2[:, :], tj2[:, :], INV_SQRT2, xd[:, :], ALU.mult,
                ALU.add), 3)

            # powers: ps = p1+p2 computed by the DMA CCE adder (off the
            # critical path), then S = sum_k via strided add tree
            ps = work_pool.tile([P, FK], BF, tag="ps")
            nc.gpsimd.dma_start(
                ps[:, :], xind[:, base + 4 * FK:base + 9 * FK // 2].bitcast(BF))
            nc.gpsimd.dma_start(
                ps[:, :], xind[:, base + 9 * FK // 2:base + 5 * FK].bitcast(BF),
                accum_op=ALU.add)
            pscr = head_pool.tile([P, 1], BF, tag="pscr")
            ps_touch = nc.vector.tensor_copy(pscr[:, :], ps[:, 0:1])
            first_pk = True
            pk = ps
            kwidth = FK
            while kwidth > TC:
                kwidth //= 2
                nk = work_pool.tile([P, kwidth], BF, tag=f"pk{kwidth}")
                inst = nc.vector.tensor_tensor(
                    nk[:, :], pk[:, 0:2 * kwidth:2], pk[:, 1:2 * kwidth:2],
                    ALU.add)
                if first_pk:
                    tile.add_dep_helper(inst.ins, ps_touch.ins, False,
                                        "toucher carries the DMA wait")
                    first_pk = False
                pk = nk
            S_t = pk

            # pairwise product w = v1*v2, as [re|im] halves
            m12 = work_pool.tile([P, 2 * FK], FP, tag="m12")
            m34 = work_pool.tile([P, 2 * FK], FP, tag="m34")
            # v2 swapped halves: [im2|re2]
            v2sw = v2[:, :].rearrange("p (two n) -> p two n", two=2)[:, ::-1, :]
            nc.vector.tensor_tensor(m12[:, :], v1[:, :], v2[:, :], ALU.mult)
            nc.vector.tensor_tensor(m34[:, :], v1[:, :], v2sw, ALU.mult)
            w = work_pool.tile([P, 2 * FK], FP, tag="v1")  # v1 dead now
            nc.vector.tensor_tensor(
                w[:, 0:FK], m12[:, 0:FK], m12[:, FK:2 * FK], ALU.subtract)
            nc.vector.tensor_tensor(
                w[:, FK:2 * FK], m34[:, 0:FK], m34[:, FK:2 * FK], ALU.add)

            # product tree over K: 8 -> 4 -> 2 -> 1.  w layout [P, 2, width]
            width = FK
            while width > TC:
                width //= 2
                wv = w[:, :].rearrange("p (two n) -> p two n", two=2)
                ev = wv[:, :, 0::2]
                ov = wv[:, :, 1::2]
                ovsw = ov[:, ::-1, :]
                q12 = work_pool.tile([P, 2 * width], FP, tag="m12")
                q34 = work_pool.tile([P, 2 * width], FP, tag="m34")
                nc.vector.tensor_tensor(
                    q12[:, :].rearrange("p (two n) -> p two n", two=2),
                    ev, ov, ALU.mult)
                nc.vector.tensor_tensor(
                    q34[:, :].rearrange("p (two n) -> p two n", two=2),
                    ev, ovsw, ALU.mult)
                nw = work_pool.tile([P, 2 * width], FP, tag="tj2")
                nc.vector.tensor_tensor(
                    nw[:, 0:width], q12[:, 0:width], q12[:, width:2 * width],
                    ALU.subtract)
                nc.vector.tensor_tensor(
                    nw[:, width:2 * width], q34[:, 0:width],
                    q34[:, width:2 * width], ALU.add)
                w = nw

            # pw = 2^S exactly: (S+127)*2^23 is an exact f32 integer; convert
            # to i32 and reinterpret the bits as f32.  Kept on the Vector
            # engine: an ACT-engine detour exposes cross-engine latency at
            # the chunk tail (measured ~5us stall).
            pq = work_pool.tile([P, TC], FP, tag="pq")
            pwi = work_pool.tile([P, TC], I32, tag="pwi")
            nc.vector.tensor_scalar(
                pq[:, :], S_t[:, :], 127.0, float(1 << 23), ALU.add, ALU.mult)
            nc.vector.tensor_copy(pwi[:, :], pq[:, :])
            pw = pwi[:, :].bitcast(FP)

            # sum_l w * 2^S -> this chunk's [re, im] accumulator columns
            # (STT with free-dim accumulate)
            dummy = work_pool.tile([P, 2 * TC], FP, tag="dummy")
            nc.vector.scalar_tensor_tensor(
                dummy[:, 0:TC], w[:, 0:TC], 1.0, pw, ALU.mult, ALU.mult,
                accum_out=acc[:, 2 * ch:2 * ch + 1])
            nc.vector.scalar_tensor_tensor(
                dummy[:, TC:2 * TC], w[:, TC:2 * TC], 1.0, pw, ALU.mult,
                ALU.mult, accum_out=acc[:, 2 * ch + 1:2 * ch + 2])

        outt = acc_pool.tile([P, 2], FP)
        nc.vector.tensor_reduce(
            outt[:, :], acc[:, :].rearrange("p (c two) -> p two c", two=2),
            AX.X, ALU.add)
        # HWDGE for the tiny result store: the SWDGE path costs a ~4us Q7
        # drain on the kernel tail waiting for the HBM write receipt.
        nc.sync.dma_start(outd[:, :], outt[:, :])


_PROGRAM = None


def _get_program():
    global _PROGRAM
    if _PROGRAM is None:
        _PROGRAM = build_program()
    return _PROGRAM


def _to_bf16_bits(a):
    """f32 array of exact small ints -> uint16 bf16 bit patterns."""
    return (np.ascontiguousarray(a, dtype=np.float32).view(np.uint32) >> 16
            ).astype(np.uint16)


def pack_core_input(c1, c2, p1, p2):
    """Pack one core's inputs into [P, RPP*K*NBLK/2] f32 (bf16 bit blocks).

    Rows (b*L+l) map to partition r//RPP, chunk (r%RPP)//TC; within a chunk
    there are NBLK dense bf16 blocks of TC*K values each:
    c1_1, c1_3, c1_0, c1_2, c2_1, c2_3, c2_0, c2_2, p1, p2."""
    u = np.empty((P, NCHUNK, NBLK, TC * K), dtype=np.uint16)

    def comp(c, j):
        return _to_bf16_bits(c[..., j]).reshape(P, NCHUNK, TC * K)

    u[:, :, 0] = comp(c1, 1)
    u[:, :, 1] = comp(c1, 3)
    u[:, :, 2] = comp(c1, 0)
    u[:, :, 3] = comp(c1, 2)
    u[:, :, 4] = comp(c2, 1)
    u[:, :, 5] = comp(c2, 3)
    u[:, :, 6] = comp(c2, 0)
    u[:, :, 7] = comp(c2, 2)
    u[:, :, 8] = _to_bf16_bits(p1.astype(np.float32)).reshape(P, NCHUNK, TC * K)
    u[:, :, 9] = _to_bf16_bits(p2.astype(np.float32)).reshape(P, NCHUNK, TC * K)
    return u.reshape(P, -1).view(np.float32)


def kernel(coeffs1, coeffs2, power1, power2):
    coeffs1 = np.asarray(coeffs1, dtype=np.float32)
    coeffs2 = np.asarray(coeffs2, dtype=np.float32)
    power1 = np.asarray(power1)
    power2 = np.asarray(power2)
    nc = _get_program()
    in_maps = []
    for ci in range(NCORES):
        sl = slice(ci * BC, (ci + 1) * BC)
        in_maps.append({
            "xin": pack_core_input(coeffs1[sl], coeffs2[sl],
                                   power1[sl], power2[sl]),
        })
    res = run_bass_kernel_spmd(nc, in_maps, core_ids=list(range(NCORES)))
    outs = []
    for ci in range(NCORES):
        o = res.results[ci]["out"]  # [128, 2]
        outs.append(o.reshape(BC, P // BC, 2).sum(axis=1, dtype=np.float32))
    return np.concatenate(outs, axis=0).astype(np.float32)


# revision 38
# speedup vs baseline: 1.1812x; 1.0470x over previous
"""Trainium2 Bass kernel for nn_ExactScalarArray.

Math: the reference computes, per (b, l):  prod_k reduce(c1*c2, p1+p2)
in an exact ring representation Z[w], w = e^{i pi/4}, then converts to
complex and sums over l with power-of-two alignment.  The ring embed
into C is a homomorphism and the reduce step is value-preserving, so
the whole thing equals

    out[b] = sum_l ( prod_k v1(b,l,k) * v2(b,l,k) ) * 2^{S(b,l)}
    v(c)   = (c0 + (c1+c3)/sqrt2) + i (c2 + (c1-c3)/sqrt2)
    S      = sum_k (p1+p2)

evaluated here in f32 complex arithmetic (max rel err vs the reference
~9e-6, measured).  Sharding: batch dim B=256 split across 8 cores; all
reduction axes (K, L) are core-local, so no collectives.

Host-side, the inputs (exact {0,1} values) are packed into dense bf16
component blocks inside one tensor per core: lossless, halves the HBM
traffic, and each chunk is a single DMA (one wait on the first
consumer; each ISA instruction has one sync-wait slot).
"""

import numpy as np

import concourse.bass as bass
import concourse.mybir as mybir
import concourse.tile as tile
from concourse.bass_utils import run_bass_kernel_spmd

# Problem shape (hardcoded per contract)
B, L, K = 256, 2048, 8
NCORES = 8
BC = B // NCORES            # 32 batch rows per core
NR = BC * L                 # 65536 (b,l) rows per core
P = 128                     # SBUF partitions
RPP = NR // P               # 512 rows per partition
TC = 256                    # rows-per-partition per chunk
NCHUNK = RPP // TC          # 2 chunks
NBLK = 12                   # bf16 blocks per chunk (each TC*K):
                            #   c1_1, c1_3, -c1_3, c1_0, c1_2,
                            #   c2_1, c2_3, -c2_3, c2_0, c2_2, p1, p2
                            # (negated blocks: the DMA CCE adder has no
                            # subtract; chunk 1 computes its t-tiles via CCE)
INV_SQRT2 = 0.7071067811865476

FP = mybir.dt.float32
BF = mybir.dt.bfloat16
I32 = mybir.dt.int32
ALU = mybir.AluOpType
AX = mybir.AxisListType

FK = TC * K               # one block, bf16 elements
FXF = TC * K * NBLK // 2  # packed chunk size in f32 units


def build_program(split_waits=True):
    nc = bass.Bass("TRN2", target_bir_lowering=False, debug=False,
                   num_devices=NCORES)
    xind = nc.dram_tensor("xin", [P, RPP * K * NBLK // 2], FP,
                          kind="ExternalInput").ap()
    outd = nc.dram_tensor("out", [P, 2], FP, kind="ExternalOutput").ap()
    with tile.TileContext(nc) as tc:
        build_kernel(nc, tc, xind, outd)
    if split_waits:
        _split_multiwait(nc)
    return nc


def _split_multiwait(nc):
    """Walrus allows one sync-wait per ISA instruction; hoist extras onto
    NOPs inserted just before the offender on the same engine."""
    k = 0
    for f in nc.m.functions:
        for bb in f.blocks:
            il = bb.instructions
            i = 0
            while i < len(il):
                inst = il[i]
                si = inst.sync_info
                if si is not None and si.on_wait and len(si.on_wait) > 1:
                    waits = list(si.on_wait)
                    for w in waits[:-1]:
                        nop = mybir.InstNoOp(name=f"WSPLIT-{k}", ins=[], outs=[])
                        k += 1
                        nop.engine = inst.engine
                        nop.sync_info = mybir.SyncInfo(on_wait=[w], on_update=[])
                        il.insert(i, nop)
                        i += 1
                    si.on_wait = waits[-1:]
                    inst.sync_info = si
                i += 1


def build_kernel(nc, tc, xind, outd):
    # With NCHUNK == 2 and double-buffered io/head pools there is no SBUF
    # slot reuse across chunks for the DMA'd tile or its first consumer, so
    # no instruction needs two sync waits (one ISA wait slot each).  Work
    # tiles are tag-aliased to fit SBUF; every alias chain is read/written
    # strictly sequentially on the Vector engine.
    with (
        tc.tile_pool(name="io", bufs=2) as io_pool,
        tc.tile_pool(name="head", bufs=2) as head_pool,
        tc.tile_pool(name="work", bufs=1) as work_pool,
        tc.tile_pool(name="acc", bufs=1) as acc_pool,
    ):
        acc = acc_pool.tile([P, 2 * NCHUNK], FP)

        for ch in range(NCHUNK):
            # five block-pair loads per chunk so compute starts after the
            # first ~1 MiB; each DMA's completion wait is absorbed by a tiny
            # "toucher" copy, so the real consumers only ever carry their
            # single WAR wait (one ISA wait slot per instruction).
            base = ch * FXF
            half = FK // 2   # one bf16 block in f32 units

            def bsrc(i, n=1):
                return xind[:, base + i * half:base + (i + n) * half]

            tj1 = head_pool.tile([P, 2 * FK], BF, tag="tj1")
            tj2 = head_pool.tile([P, 2 * FK], BF, tag="tj2")
            v1 = work_pool.tile([P, 2 * FK], FP, tag="v1")
            v2 = work_pool.tile([P, 2 * FK], FP, tag="v2")
            touches = {}

            def touch(nm, t):
                scr = head_pool.tile([P, 1], t.dtype, tag=f"scr_{nm}")
                touches[nm] = nc.vector.tensor_copy(scr[:, :], t[:, 0:1])

            def after_touch(inst, nm):
                tile.add_dep_helper(inst.ins, touches[nm].ins, False,
                                    "toucher carries the DMA wait")
                return inst

            if ch == 0:
                # ramp-critical chunk: plain loads, t = c1 +- c3 on the DVE
                xa = io_pool.tile([P, FK], FP, tag="xa")
                xc = io_pool.tile([P, FK], FP, tag="xc")
                xb = io_pool.tile([P, FK], FP, tag="xb")
                xd = io_pool.tile([P, FK], FP, tag="xd")
                nc.gpsimd.dma_start(xa[:, :], bsrc(0, 2))
                nc.gpsimd.dma_start(xc[:, :], bsrc(5, 2))
                nc.gpsimd.dma_start(xb[:, :], bsrc(3, 2))
                nc.gpsimd.dma_start(xd[:, :], bsrc(8, 2))
                for nm, t in (("xa", xa), ("xc", xc), ("xb", xb), ("xd", xd)):
                    touch(nm, t)
                xab, xcb = xa[:, :].bitcast(BF), xc[:, :].bitcast(BF)
                after_touch(nc.vector.tensor_tensor(
                    tj1[:, 0:FK], xab[:, 0:FK], xab[:, FK:2 * FK], ALU.add),
                    "xa")
                after_touch(nc.vector.tensor_tensor(
                    tj1[:, FK:2 * FK], xab[:, 0:FK], xab[:, FK:2 * FK],
                    ALU.subtract), "xa")
                after_touch(nc.vector.tensor_tensor(
                    tj2[:, 0:FK], xcb[:, 0:FK], xcb[:, FK:2 * FK], ALU.add),
                    "xc")
                after_touch(nc.vector.tensor_tensor(
                    tj2[:, FK:2 * FK], xcb[:, 0:FK], xcb[:, FK:2 * FK],
                    ALU.subtract), "xc")
                stt1_dep, stt2_dep = "xb", "xd"
            else:
                # steady-state chunk: DVE is busy with the previous chunk, so
                # the DMA CCE adder computes the t-tiles during the loads
                # (base copy then accumulate; receipt latency fully hidden)
                xb = io_pool.tile([P, FK], FP, tag="xb")
                xd = io_pool.tile([P, FK], FP, tag="xd")
                nc.gpsimd.dma_start(tj1[:, 0:FK], bsrc(0).bitcast(BF))
                nc.gpsimd.dma_start(tj1[:, FK:2 * FK], bsrc(0).bitcast(BF))
                nc.gpsimd.dma_start(xb[:, :], bsrc(3, 2))
                nc.gpsimd.dma_start(tj1[:, 0:FK], bsrc(1).bitcast(BF),
                                    accum_op=ALU.add)
                nc.gpsimd.dma_start(tj1[:, FK:2 * FK], bsrc(2).bitcast(BF),
                                    accum_op=ALU.add)
                nc.gpsimd.dma_start(tj2[:, 0:FK], bsrc(5).bitcast(BF))
                nc.gpsimd.dma_start(tj2[:, FK:2 * FK], bsrc(5).bitcast(BF))
                nc.gpsimd.dma_start(xd[:, :], bsrc(8, 2))
                nc.gpsimd.dma_start(tj2[:, 0:FK], bsrc(6).bitcast(BF),
                                    accum_op=ALU.add)
                nc.gpsimd.dma_start(tj2[:, FK:2 * FK], bsrc(7).bitcast(BF),
                                    accum_op=ALU.add)
                for nm, t in (("tj1", tj1), ("tj2", tj2), ("xb", xb),
                              ("xd", xd)):
                    touch(nm, t)
                stt1_dep, stt2_dep = "tj1", "tj2"

            after_touch(nc.vector.scalar_tensor_tensor(
                v1[:, :], tj1[:, :], INV_SQRT2, xb[:, :].bitcast(BF),
                ALU.mult, ALU.add), stt1_dep)
            after_touch(nc.vector.scalar_tensor_tensor(
                v2[:, :], tj2[:, :], INV_SQRT2, xd[:, :].bitcast(BF),
                ALU.mult, ALU.add), stt2_dep)

            # powers: ps = p1+p2 computed by the DMA CCE adder (off the
            # critical path), then S = sum_k via strided add tree
            ps = work_pool.tile([P, FK], BF, tag="ps")
            nc.gpsimd.dma_start(ps[:, :], bsrc(10).bitcast(BF))
            nc.gpsimd.dma_start(ps[:, :], bsrc(11).bitcast(BF),
                                accum_op=ALU.add)
            touch("ps", ps)
            ps_touch = touches["ps"]
            first_pk = True
            pk = ps
            kwidth = FK
            while kwidth > TC:
                kwidth //= 2
                nk = work_pool.tile([P, kwidth], BF, tag=f"pk{kwidth}")
                inst = nc.vector.tensor_tensor(
                    nk[:, :], pk[:, 0:2 * kwidth:2], pk[:, 1:2 * kwidth:2],
                    ALU.add)
                if first_pk:
                    tile.add_dep_helper(inst.ins, ps_touch.ins, False,
                                        "toucher carries the DMA wait")
                    first_pk = False
                pk = nk
            S_t = pk

            # pairwise product w = v1*v2, as [re|im] halves
            m12 = work_pool.tile([P, 2 * FK], FP, tag="m12")
            m34 = work_pool.tile([P, 2 * FK], FP, tag="m34")
            # v2 swapped halves: [im2|re2]
            v2sw = v2[:, :].rearrange("p (two n) -> p two n", two=2)[:, ::-1, :]
            nc.vector.tensor_tensor(m12[:, :], v1[:, :], v2[:, :], ALU.mult)
            nc.vector.tensor_tensor(m34[:, :], v1[:, :], v2sw, ALU.mult)
            w = work_pool.tile([P, 2 * FK], FP, tag="v1")  # v1 dead now
            nc.vector.tensor_tensor(
                w[:, 0:FK], m12[:, 0:FK], m12[:, FK:2 * FK], ALU.subtract)
            nc.vector.tensor_tensor(
                w[:, FK:2 * FK], m34[:, 0:FK], m34[:, FK:2 * FK], ALU.add)

            # product tree over K: 8 -> 4 -> 2 -> 1.  w layout [P, 2, width]
            width = FK
            while width > TC:
                width //= 2
                wv = w[:, :].rearrange("p (two n) -> p two n", two=2)
                ev = wv[:, :, 0::2]
                ov = wv[:, :, 1::2]
                ovsw = ov[:, ::-1, :]
                q12 = work_pool.tile([P, 2 * width], FP, tag="m12")
                q34 = work_pool.tile([P, 2 * width], FP, tag="m34")
                nc.vector.tensor_tensor(
                    q12[:, :].rearrange("p (two n) -> p two n", two=2),
                    ev, ov, ALU.mult)
                nc.vector.tensor_tensor(
                    q34[:, :].rearrange("p (two n) -> p two n", two=2),
                    ev, ovsw, ALU.mult)
                nw = work_pool.tile([P, 2 * width], FP, tag="tj2")
                nc.vector.tensor_tensor(
                    nw[:, 0:width], q12[:, 0:width], q12[:, width:2 * width],
                    ALU.subtract)
                nc.vector.tensor_tensor(
                    nw[:, width:2 * width], q34[:, 0:width],
                    q34[:, width:2 * width], ALU.add)
                w = nw

            # pw = 2^S exactly: (S+127)*2^23 is an exact f32 integer; convert
            # to i32 and reinterpret the bits as f32.  Kept on the Vector
            # engine: an ACT-engine detour exposes cross-engine latency at
            # the chunk tail (measured ~5us stall).
            pq = work_pool.tile([P, TC], FP, tag="pq")
            pwi = work_pool.tile([P, TC], I32, tag="pwi")
            nc.vector.tensor_scalar(
                pq[:, :], S_t[:, :], 127.0, float(1 << 23), ALU.add, ALU.mult)
            nc.vector.tensor_copy(pwi[:, :], pq[:, :])
            pw = pwi[:, :].bitcast(FP)

            # sum_l w * 2^S -> this chunk's [re, im] accumulator columns
            # (STT with free-dim accumulate)
            dummy = work_pool.tile([P, 2 * TC], FP, tag="dummy")
            nc.vector.scalar_tensor_tensor(
                dummy[:, 0:TC], w[:, 0:TC], 1.0, pw, ALU.mult, ALU.mult,
                accum_out=acc[:, 2 * ch:2 * ch + 1])
            nc.vector.scalar_tensor_tensor(
                dummy[:, TC:2 * TC], w[:, TC:2 * TC], 1.0, pw, ALU.mult,
                ALU.mult, accum_out=acc[:, 2 * ch + 1:2 * ch + 2])

        outt = acc_pool.tile([P, 2], FP)
        nc.vector.tensor_reduce(
            outt[:, :], acc[:, :].rearrange("p (c two) -> p two c", two=2),
            AX.X, ALU.add)
        # HWDGE for the tiny result store: the SWDGE path costs a ~4us Q7
        # drain on the kernel tail waiting for the HBM write receipt.
        nc.sync.dma_start(outd[:, :], outt[:, :])


_PROGRAM = None


def _get_program():
    global _PROGRAM
    if _PROGRAM is None:
        _PROGRAM = build_program()
    return _PROGRAM


def _to_bf16_bits(a):
    """f32 array of exact small ints -> uint16 bf16 bit patterns."""
    return (np.ascontiguousarray(a, dtype=np.float32).view(np.uint32) >> 16
            ).astype(np.uint16)


def pack_core_input(c1, c2, p1, p2):
    """Pack one core's inputs into [P, RPP*K*NBLK/2] f32 (bf16 bit blocks).

    Rows (b*L+l) map to partition r//RPP, chunk (r%RPP)//TC; within a chunk
    there are NBLK dense bf16 blocks of TC*K values each:
    c1_1, c1_3, c1_0, c1_2, c2_1, c2_3, c2_0, c2_2, p1, p2."""
    u = np.empty((P, NCHUNK, NBLK, TC * K), dtype=np.uint16)

    def comp(c, j):
        return _to_bf16_bits(c[..., j]).reshape(P, NCHUNK, TC * K)

    u[:, :, 0] = comp(c1, 1)
    u[:, :, 1] = comp(c1, 3)
    u[:, :, 2] = _to_bf16_bits(-c1[..., 3]).reshape(P, NCHUNK, TC * K)
    u[:, :, 3] = comp(c1, 0)
    u[:, :, 4] = comp(c1, 2)
    u[:, :, 5] = comp(c2, 1)
    u[:, :, 6] = comp(c2, 3)
    u[:, :, 7] = _to_bf16_bits(-c2[..., 3]).reshape(P, NCHUNK, TC * K)
    u[:, :, 8] = comp(c2, 0)
    u[:, :, 9] = comp(c2, 2)
    u[:, :, 10] = _to_bf16_bits(p1.astype(np.float32)).reshape(P, NCHUNK, TC * K)
    u[:, :, 11] = _to_bf16_bits(p2.astype(np.float32)).reshape(P, NCHUNK, TC * K)
    return u.reshape(P, -1).view(np.float32)


def kernel(coeffs1, coeffs2, power1, power2):
    coeffs1 = np.asarray(coeffs1, dtype=np.float32)
    coeffs2 = np.asarray(coeffs2, dtype=np.float32)
    power1 = np.asarray(power1)
    power2 = np.asarray(power2)
    nc = _get_program()
    in_maps = []
    for ci in range(NCORES):
        sl = slice(ci * BC, (ci + 1) * BC)
        in_maps.append({
            "xin": pack_core_input(coeffs1[sl], coeffs2[sl],
                                   power1[sl], power2[sl]),
        })
    res = run_bass_kernel_spmd(nc, in_maps, core_ids=list(range(NCORES)))
    outs = []
    for ci in range(NCORES):
        o = res.results[ci]["out"]  # [128, 2]
        outs.append(o.reshape(BC, P // BC, 2).sum(axis=1, dtype=np.float32))
    return np.concatenate(outs, axis=0).astype(np.float32)
